# revision 17
# baseline (speedup 1.0000x reference)
"""Trainium2 Bass kernel for nn_AttnCodebook (VQ codebook attention block).

Sharding: data-parallel over batch B=8 -> one batch element per NeuronCore.
Each core computes its batch element's attention + FF block; q/k/logits run
in a transposed activation layout, softmax rows in natural layout.

Device outputs per core: log_alpha_tau [N, C] f32 and codebook_output [N, D]
f32.  log_alpha and z are reconstructed on the host from log_alpha_tau
(exact linear relation / softmax), which halves the device DMA-out traffic.

Precision: float32r (TF32-like, full PE rate) for the rmsnorm->q/k->logits
chain; bf16 for the attention value path, Wc and the feed-forward.
"""

import math
import numpy as np

import concourse.bacc as bacc
import concourse.tile as tile
from concourse import mybir, masks
from concourse.bass_utils import run_bass_kernel_spmd
from contextlib import ExitStack

B, N, D, C, H = 8, 2048, 768, 2048, 3072
HALF = D // 2  # 384
ND = D // 128  # 6
NH = H // 128  # 24
NCC = C // 128  # 16
NNC = N // 128  # 16
NBLK = 4  # n-blocks of 512
BLK = N // NBLK  # 512
ROPE_BASE = 10000.0
EPS = 1e-6

dt = mybir.dt
AF = mybir.ActivationFunctionType
OP = mybir.AluOpType

F32, F32R, BF16 = dt.float32, dt.float32r, dt.bfloat16


def build_program(rs: float):
    """Build the per-core Bass program (same program for all 8 cores)."""
    nc = bacc.Bacc("TRN2", target_bir_lowering=False, debug=False, num_devices=1)

    # ---- DRAM I/O ----
    lat = nc.dram_tensor("lat", [N, D], F32, kind="ExternalInput").ap()
    cb = nc.dram_tensor("cb", [C, D], F32, kind="ExternalInput").ap()
    g2 = nc.dram_tensor("g2", [N, C], F32, kind="ExternalInput").ap()
    cosq = nc.dram_tensor("cosq", [HALF, N], F32, kind="ExternalInput").ap()
    sinq = nc.dram_tensor("sinq", [HALF, N], F32, kind="ExternalInput").ap()
    wq = nc.dram_tensor("wq", [D, D], F32R, kind="ExternalInput").ap()
    wk = nc.dram_tensor("wk", [D, D], F32R, kind="ExternalInput").ap()
    wv = nc.dram_tensor("wv", [D, D], F32R, kind="ExternalInput").ap()
    bv = nc.dram_tensor("bv", [1, D], F32R, kind="ExternalInput").ap()
    wc = nc.dram_tensor("wc", [D, D], BF16, kind="ExternalInput").ap()
    w1 = nc.dram_tensor("w1", [D, H], BF16, kind="ExternalInput").ap()
    w3 = nc.dram_tensor("w3", [D, H], BF16, kind="ExternalInput").ap()
    w2 = nc.dram_tensor("w2", [H, D], BF16, kind="ExternalInput").ap()

    latau = nc.dram_tensor("latau", [N, C], F32, kind="ExternalOutput").ap()
    outp = nc.dram_tensor("outp", [N, D], F32, kind="ExternalOutput").ap()
    aos = nc.dram_tensor("aos", [N, D], BF16, kind="ExternalOutput").ap()  # attn_out spill

    with tile.TileContext(nc) as tc, ExitStack() as octx:
        sbG = octx.enter_context(tc.tile_pool(name="glob", bufs=1))
        sbSc = octx.enter_context(tc.tile_pool(name="scal", bufs=4))
        psB = octx.enter_context(tc.tile_pool(name="psB", bufs=2, space="PSUM"))
        psAV = octx.enter_context(tc.tile_pool(name="psAV", bufs=1, space="PSUM"))
        psT = octx.enter_context(tc.tile_pool(name="psT", bufs=2, space="PSUM"))

        idf = sbG.tile([128, 128], F32, tag="idf")
        masks.make_identity(nc, idf[:])
        idb = sbG.tile([128, 128], BF16, tag="idb")
        masks.make_identity(nc, idb[:])
        ones_f = sbG.tile([1, 128], F32, tag="ones_f")
        nc.gpsimd.memset(ones_f[:], 1.0)
        ones_r = sbG.tile([1, 128], F32R, tag="ones_r")
        nc.vector.tensor_copy(ones_r[:], ones_f[:])
        eps_t = sbG.tile([128, 1], F32, tag="eps_t")
        nc.gpsimd.memset(eps_t[:], EPS)

        def rms_rinv(pool, x_t, nfree, tagp):
            """per-partition 1/sqrt(mean(x^2)+eps) of a [128, nfree] tile."""
            sq = pool.tile([128, nfree], F32, tag=f"sq{tagp}", bufs=1, name="sq")
            ss = sbSc.tile([128, 1], F32, tag=f"ss{tagp}", name="ss")
            nc.scalar.activation(sq[:], x_t[:], AF.Square, accum_out=ss[:])
            rms = sbSc.tile([128, 1], F32, tag=f"rms{tagp}", name="rms")
            nc.scalar.activation(rms[:], ss[:], AF.Sqrt, scale=1.0 / nfree,
                                 bias=eps_t[:])
            rinv = sbSc.tile([128, 1], F32, tag=f"rinv{tagp}", name="rinv")
            nc.vector.reciprocal(rinv[:], rms[:])
            return rinv

        def rope_pair(pool, ps_lo, ps_hi, cs_t, sn_t, out_lo, out_hi, w, tagp):
            """out_lo = lo*cos - hi*sin ; out_hi = lo*sin + hi*cos (width w)."""
            t1 = pool.tile([128, w], F32, tag=f"t1{tagp}", bufs=1, name="t1")
            t2 = pool.tile([128, w], F32, tag=f"t2{tagp}", bufs=1, name="t2")
            nc.vector.tensor_mul(t1[:], ps_lo, cs_t)
            nc.vector.tensor_mul(t2[:], ps_hi, sn_t)
            nc.vector.tensor_tensor(out_lo, t1[:], t2[:], OP.subtract)
            nc.vector.tensor_mul(t1[:], ps_lo, sn_t)
            nc.vector.tensor_mul(t2[:], ps_hi, cs_t)
            nc.vector.tensor_tensor(out_hi, t1[:], t2[:], OP.add)

        with tc.tile_pool(name="resA", bufs=1) as sbA:
            kT = [sbA.tile([128, C], F32R, tag=f"kT{j}", name=f"kT{j}")
                  for j in range(ND)]
            vT = [sbA.tile([128, D], BF16, tag=f"v{j}", name=f"v{j}")
                  for j in range(NCC)]
            wc_t = [sbA.tile([128, D], BF16, tag=f"wc{j}", name=f"wc{j}")
                    for j in range(ND)]
            wq_t = [sbA.tile([128, D], F32R, tag=f"wq{j}", name=f"wq{j}")
                    for j in range(ND)]

            # ===== PHASE 0: codebook -> kT (roped f32r), v (bf16), by C-half ====
            with tc.tile_pool(name="ph0", bufs=2) as sb0:
                wk_t = [sb0.tile([128, D], F32R, tag=f"wk{j}", bufs=1,
                                 name=f"wk{j}") for j in range(ND)]
                wv_t = [sb0.tile([128, D], F32R, tag=f"wv{j}", bufs=1,
                                 name=f"wv{j}") for j in range(ND)]
                for j in range(ND):
                    nc.sync.dma_start(wk_t[j][:], wk[j * 128:(j + 1) * 128, :])
                bv_t = sb0.tile([1, D], F32R, tag="bv", bufs=1)
                nc.sync.dma_start(bv_t[:], bv[:])

                for hh in range(2):
                    c0 = hh * 1024
                    ncsT = [sb0.tile([128, 1024], F32R, tag=f"ncsT{j}", bufs=1,
                                     name=f"ncsT{j}") for j in range(ND)]
                    for i in range(8):
                        ci = hh * 8 + i
                        cb_t = sb0.tile([128, D], F32, tag="cb", name="cb")
                        nc.sync.dma_start(cb_t[:], cb[ci * 128:(ci + 1) * 128, :])
                        rinv = rms_rinv(sb0, cb_t, D, "cb")
                        ncs = sb0.tile([128, D], F32, tag="ncs", name="ncs")
                        nc.vector.tensor_scalar_mul(ncs[:], cb_t[:], rinv[:])
                        for j in range(ND):
                            pt = psT.tile([128, 128], F32, tag="tp", name="pt")
                            nc.tensor.transpose(pt[:], ncs[:, j * 128:(j + 1) * 128],
                                                idf[:])
                            nc.scalar.copy(ncsT[j][:, i * 128:(i + 1) * 128], pt[:])
                    # k projection + rope for this half
                    for j in range(ND // 2):
                        pslo = psB.tile([128, 1024], F32, tag="mm", name="pslo")
                        pshi = psB.tile([128, 1024], F32, tag="mm", name="pshi")
                        for ps, jj in ((pslo, j), (pshi, j + ND // 2)):
                            for dk in range(ND):
                                for s in range(2):
                                    nc.tensor.matmul(
                                        ps[:, s * 512:(s + 1) * 512],
                                        wk_t[dk][:, jj * 128:(jj + 1) * 128],
                                        ncsT[dk][:, s * 512:(s + 1) * 512],
                                        start=(dk == 0), stop=(dk == ND - 1))
                        cs_t = sb0.tile([128, 1024], F32, tag="cosk", name="cs")
                        sn_t = sb0.tile([128, 1024], F32, tag="sink", name="sn")
                        nc.sync.dma_start(cs_t[:], cosq[j * 128:(j + 1) * 128,
                                                        c0:c0 + 1024])
                        nc.sync.dma_start(sn_t[:], sinq[j * 128:(j + 1) * 128,
                                                        c0:c0 + 1024])
                        rope_pair(sb0, pslo[:], pshi[:], cs_t[:], sn_t[:],
                                  kT[j][:, c0:c0 + 1024],
                                  kT[j + ND // 2][:, c0:c0 + 1024], 1024, "k")
                    # v for this half
                    if hh == 0:
                        for j in range(ND):
                            nc.sync.dma_start(wv_t[j][:], wv[j * 128:(j + 1) * 128, :])
                    for i in range(8):
                        ci = hh * 8 + i
                        vps = psAV.tile([128, D], F32, tag="av", name="vps")
                        for dk in range(ND):
                            for s0, sw in ((0, 512), (512, 256)):
                                nc.tensor.matmul(
                                    vps[:, s0:s0 + sw],
                                    ncsT[dk][:, i * 128:(i + 1) * 128],
                                    wv_t[dk][:, s0:s0 + sw],
                                    start=(dk == 0), stop=False)
                        for s0, sw in ((0, 512), (512, 256)):
                            nc.tensor.matmul(vps[:, s0:s0 + sw],
                                             ones_r[:], bv_t[:, s0:s0 + sw],
                                             start=False, stop=True)
                        nc.scalar.copy(vT[ci][:], vps[:, 0:D])

            # ======== PHASE A: queries + logits + attention, per n-block ========
            with tc.tile_pool(name="phA", bufs=2) as sbW, \
                 tc.tile_pool(name="phAq", bufs=1) as sbQ:
                pending = None
                lat_all = []
                for j in range(ND):
                    nc.sync.dma_start(wq_t[j][:], wq[j * 128:(j + 1) * 128, :])
                    nc.sync.dma_start(wc_t[j][:], wc[j * 128:(j + 1) * 128, :])
                for blk in range(NBLK):
                    # -- queries for this block --
                    lat_t = [sbW.tile([128, D], F32, tag=f"lat{k}", bufs=2,
                                      name=f"lat{k}") for k in range(4)]
                    lat_all.append(lat_t)
                    nqT = [sbQ.tile([128, BLK], F32R, tag=f"nqT{j}",
                                    name=f"nqT{j}") for j in range(ND)]
                    for k in range(4):
                        nch = blk * 4 + k
                        nc.sync.dma_start(lat_t[k][:],
                                          lat[nch * 128:(nch + 1) * 128, :])
                        rinv = rms_rinv(sbW, lat_t[k], D, "q")
                        nq = sbW.tile([128, D], F32, tag="nq", bufs=1, name="nq")
                        nc.vector.tensor_scalar_mul(nq[:], lat_t[k][:], rinv[:])
                        for j in range(ND):
                            pt = psT.tile([128, 128], F32, tag="tp", name="pt")
                            nc.tensor.transpose(pt[:], nq[:, j * 128:(j + 1) * 128],
                                                idf[:])
                            nc.scalar.copy(nqT[j][:, k * 128:(k + 1) * 128], pt[:])
                    qT = [sbQ.tile([128, BLK], F32R, tag=f"qT{j}", name=f"qT{j}")
                          for j in range(ND)]
                    for j in range(ND // 2):
                        pslo = psB.tile([128, 1024], F32, tag="mm", name="pslo")
                        pshi = psB.tile([128, 1024], F32, tag="mm", name="pshi")
                        for ps, jj in ((pslo, j), (pshi, j + ND // 2)):
                            for dk in range(ND):
                                nc.tensor.matmul(
                                    ps[:, 0:BLK],
                                    wq_t[dk][:, jj * 128:(jj + 1) * 128],
                                    nqT[dk][:], start=(dk == 0), stop=(dk == ND - 1))
                        cs_t = sbW.tile([128, BLK], F32, tag="coss", bufs=1,
                                        name="cs")
                        sn_t = sbW.tile([128, BLK], F32, tag="sins", bufs=1,
                                        name="sn")
                        nc.sync.dma_start(cs_t[:], cosq[j * 128:(j + 1) * 128,
                                                        blk * BLK:(blk + 1) * BLK])
                        nc.sync.dma_start(sn_t[:], sinq[j * 128:(j + 1) * 128,
                                                        blk * BLK:(blk + 1) * BLK])
                        rope_pair(sbW, pslo[:, 0:BLK], pshi[:, 0:BLK], cs_t[:],
                                  sn_t[:], qT[j][:], qT[j + ND // 2][:], BLK, "q")

                    # -- per n-chunk: logits then (pipelined) softmax+attention --
                    def emit_logits(nch, qT):
                        lt = sbW.tile([128, C], F32, tag="latau", name="lt")
                        kk = nch % 4
                        for hh in range(2):
                            g2_t = sbW.tile([128, 1024], F32, tag="g2", bufs=2,
                                            name="g2t")
                            nc.sync.dma_start(
                                g2_t[:], g2[nch * 128:(nch + 1) * 128,
                                            hh * 1024:(hh + 1) * 1024])
                            ps = psB.tile([128, 1024], F32, tag="mm", name="laps")
                            for dk in range(ND):
                                for s in range(2):
                                    cs0 = hh * 1024 + s * 512
                                    nc.tensor.matmul(
                                        ps[:, s * 512:(s + 1) * 512],
                                        qT[dk][:, kk * 128:(kk + 1) * 128],
                                        kT[dk][:, cs0:cs0 + 512],
                                        start=(dk == 0), stop=(dk == ND - 1))
                            nc.vector.scalar_tensor_tensor(
                                lt[:, hh * 1024:(hh + 1) * 1024], ps[:], 1.0,
                                g2_t[:], OP.mult, OP.add)
                        nc.sync.dma_start(latau[nch * 128:(nch + 1) * 128, :], lt[:])
                        # row-sum of exp via fused accumulate (e image unused)
                        e_bf = sbW.tile([128, C], BF16, tag="e", bufs=1, name="e_bf")
                        ssum = sbSc.tile([128, 1], F32, tag="ssum", name="ssum")
                        nc.scalar.activation(e_bf[:], lt[:], AF.Exp,
                                             accum_out=ssum[:])
                        rr = sbSc.tile([128, 1], F32, tag="rr", name="rr")
                        nc.vector.reciprocal(rr[:], ssum[:])
                        return nch, lt, rr

                    def emit_attn(state):
                        nch, lt, rr = state
                        kk = nch % 4
                        # eT tiles: transpose la_tau then exp on ACT (evacuates psT)
                        eTs = []
                        for ci in range(NCC):
                            pt = psT.tile([128, 128], F32, tag="tp", name="pt")
                            nc.tensor.transpose(pt[:], lt[:, ci * 128:(ci + 1) * 128],
                                                idf[:])
                            eT = sbW.tile([128, 128], BF16, tag=f"eT{ci}", bufs=1,
                                          name="eT")
                            nc.scalar.activation(eT[:], pt[:], AF.Exp)
                            eTs.append(eT)
                        aps = psAV.tile([128, D], F32, tag="av", name="aps")
                        for ci in range(NCC):
                            for s0, sw in ((0, 512), (512, 256)):
                                nc.tensor.matmul(aps[:, s0:s0 + sw], eTs[ci][:],
                                                 vT[ci][:, s0:s0 + sw],
                                                 start=(ci == 0), stop=(ci == NCC - 1))
                        at_bf = sbW.tile([128, D], BF16, tag="at", name="at")
                        nc.vector.tensor_scalar_mul(at_bf[:], aps[:, 0:D], rr[:])
                        yps = psAV.tile([128, D], F32, tag="av", name="yps")
                        aTs = []
                        for j in range(ND):
                            pt = psT.tile([128, 128], BF16, tag="tp", name="pt")
                            nc.tensor.transpose(pt[:], at_bf[:, j * 128:(j + 1) * 128],
                                                idb[:])
                            aT = sbW.tile([128, 128], BF16, tag=f"aT{j}", bufs=1,
                                          name="aT")
                            nc.scalar.copy(aT[:], pt[:])
                            aTs.append(aT)
                        for j in range(ND):
                            for s0, sw in ((0, 512), (512, 256)):
                                nc.tensor.matmul(yps[:, s0:s0 + sw], aTs[j][:],
                                                 wc_t[j][:, s0:s0 + sw],
                                                 start=(j == 0), stop=(j == ND - 1))
                        ao_bf = sbW.tile([128, D], BF16, tag="ao", name="ao")
                        nc.vector.scalar_tensor_tensor(ao_bf[:],
                                                       lat_all[nch // 4][nch % 4][:],
                                                       rs, yps[:, 0:D],
                                                       OP.mult, OP.add)
                        nc.sync.dma_start(aos[nch * 128:(nch + 1) * 128, :], ao_bf[:])

                    for k in range(4):
                        st = emit_logits(blk * 4 + k, qT)
                        if pending is not None:
                            emit_attn(pending)
                        pending = st
                if pending is not None:
                    emit_attn(pending)

        # ================= PHASE B: feed-forward =================
        with tc.tile_pool(name="phB", bufs=2) as sbB:
            # phase A writes `aos` via DRAM; Tile does not track DRAM RAW
            # deps across the spill, so order phases explicitly.
            tc.strict_bb_all_engine_barrier()

            def emit_w2(state):
                blkp, ao_p, p_p = state
                for k in range(4):
                    nch = blkp * 4 + k
                    fps = psAV.tile([128, D], F32, tag="av", name="fps")
                    for hs in range(NH):
                        for s0, sw in ((0, 512), (512, 256)):
                            nc.tensor.matmul(fps[:, s0:s0 + sw],
                                             p_p[hs][:, k * 128:(k + 1) * 128],
                                             w2keep[hs][:, s0:s0 + sw],
                                             start=(hs == 0), stop=(hs == NH - 1))
                    ot = sbB.tile([128, D], F32, tag="ot", name="ot")
                    nc.vector.scalar_tensor_tensor(ot[:], ao_p[k][:], 1.0,
                                                   fps[:, 0:D], OP.mult, OP.add)
                    nc.sync.dma_start(outp[nch * 128:(nch + 1) * 128, :], ot[:])

            # w2 is small (4.5MB bf16): keep resident, loaded once
            w2keep = [sbB.tile([128, D], BF16, tag=f"w2k{j}", bufs=1,
                               name=f"w2k{j}") for j in range(NH)]
            for j in range(NH):
                nc.sync.dma_start(w2keep[j][:], w2[j * 128:(j + 1) * 128, :])

            pendB = None
            for blk in range(NBLK):
                ao_t = [sbB.tile([128, D], BF16, tag=f"aob{k}", bufs=2,
                                 name=f"aob{k}") for k in range(4)]
                hT = [sbB.tile([128, BLK], BF16, tag=f"hT{j}", bufs=2,
                               name=f"hT{j}") for j in range(ND)]
                for k in range(4):
                    nch = blk * 4 + k
                    nc.sync.dma_start(ao_t[k][:], aos[nch * 128:(nch + 1) * 128, :])
                    rinv = rms_rinv(sbB, ao_t[k], D, "ff")
                    h_bf = sbB.tile([128, D], BF16, tag="h", name="h_bf")
                    nc.vector.tensor_scalar_mul(h_bf[:], ao_t[k][:], rinv[:])
                    for j in range(ND):
                        pt = psT.tile([128, 128], BF16, tag="tp", name="pt")
                        nc.tensor.transpose(pt[:], h_bf[:, j * 128:(j + 1) * 128],
                                            idb[:])
                        nc.scalar.copy(hT[j][:, k * 128:(k + 1) * 128], pt[:])
                p_t = [sbB.tile([128, BLK], BF16, tag=f"p{hs}", bufs=2,
                                name=f"p{hs}") for hs in range(NH)]
                for hp in range(NH // 2):
                    w1s = [sbB.tile([128, 256], BF16, tag=f"w1s{dk}", bufs=2,
                                    name="w1s") for dk in range(ND)]
                    w3s = [sbB.tile([128, 256], BF16, tag=f"w3s{dk}", bufs=2,
                                    name="w3s") for dk in range(ND)]
                    for dk in range(ND):
                        nc.sync.dma_start(
                            w1s[dk][:], w1[dk * 128:(dk + 1) * 128,
                                           hp * 256:(hp + 1) * 256])
                        nc.sync.dma_start(
                            w3s[dk][:], w3[dk * 128:(dk + 1) * 128,
                                           hp * 256:(hp + 1) * 256])
                    for hi in range(2):
                        hs = hp * 2 + hi
                        a1 = psB.tile([128, 1024], F32, tag="mm", name="a1")
                        for dk in range(ND):
                            nc.tensor.matmul(a1[:, 0:BLK],
                                             w1s[dk][:, hi * 128:(hi + 1) * 128],
                                             hT[dk][:], start=(dk == 0),
                                             stop=(dk == ND - 1))
                        a1s = sbB.tile([128, BLK], BF16, tag="a1s", name="a1s")
                        nc.scalar.activation(a1s[:], a1[:, 0:BLK], AF.Silu)
                        a3 = psB.tile([128, 1024], F32, tag="mm", name="a3")
                        for dk in range(ND):
                            nc.tensor.matmul(a3[:, 0:BLK],
                                             w3s[dk][:, hi * 128:(hi + 1) * 128],
                                             hT[dk][:], start=(dk == 0),
                                             stop=(dk == ND - 1))
                        nc.vector.tensor_tensor(p_t[hs][:], a3[:, 0:BLK], a1s[:],
                                                OP.mult)
                if pendB is not None:
                    emit_w2(pendB)
                pendB = (blk, ao_t, p_t)
            if pendB is not None:
                emit_w2(pendB)

    nc.compile()
    return nc


_PROG_CACHE = {}


def _get_program(rs: float):
    key = float(rs)
    if key not in _PROG_CACHE:
        _PROG_CACHE[key] = build_program(key)
    return _PROG_CACHE[key]


def _host_prep(latents, tau, residual_scaling, gumbel_noise_scale, positions,
               codebook, norm_context_scale, norm_queries_scale, norm_ff_scale,
               Wq, Wk, Wv, bv, Wc, W1, W2, W3):
    import ml_dtypes
    f32 = np.float32
    tau = float(np.asarray(tau))
    gns = float(np.asarray(gumbel_noise_scale))
    rs = float(np.asarray(residual_scaling))
    latents = np.asarray(latents, f32)
    positions = np.asarray(positions)
    codebook = np.ascontiguousarray(np.asarray(codebook, f32))
    s_q = np.asarray(norm_queries_scale, f32)
    s_c = np.asarray(norm_context_scale, f32)
    s_f = np.asarray(norm_ff_scale, f32)
    Wq = np.asarray(Wq, f32); Wk = np.asarray(Wk, f32); Wv = np.asarray(Wv, f32)
    bv = np.asarray(bv, f32); Wc = np.asarray(Wc, f32)
    W1 = np.asarray(W1, f32); W2 = np.asarray(W2, f32); W3 = np.asarray(W3, f32)

    # gumbel noise, bit-identical to the reference (jax threefry on CPU)
    import jax
    import jax.numpy as jnp
    cpu = jax.devices("cpu")[0]
    with jax.default_device(cpu):
        u = np.asarray(jax.random.uniform(jax.random.key(42), (B, N, C),
                                          dtype=jnp.float32))
    gum = -np.log(-np.log(u + f32(1e-10)) + f32(1e-10)).astype(f32)
    g2 = (f32(gns / tau) * gum).astype(f32)

    # rope tables, transposed per batch: [B, HALF, N]
    inv_freq = (1.0 / (ROPE_BASE ** (np.arange(HALF, dtype=f32) / f32(HALF)))).astype(f32)
    ang = positions[:, :N].astype(f32)[:, :, None] * inv_freq[None, None, :]
    cosq = np.ascontiguousarray(np.cos(ang).astype(f32).transpose(0, 2, 1))
    sinq = np.ascontiguousarray(np.sin(ang).astype(f32).transpose(0, 2, 1))

    # weight folds (host-side, exactly linear)
    wq_h = np.ascontiguousarray((Wq.T * (s_q[:, None] / f32(math.sqrt(D) * tau))).astype(f32))
    wk_h = np.ascontiguousarray((Wk.T * s_c[:, None]).astype(f32))
    wv_h = np.ascontiguousarray((Wv.T * s_c[:, None]).astype(f32))
    wc_h = np.ascontiguousarray(Wc.T.astype(ml_dtypes.bfloat16))
    w1_h = np.ascontiguousarray((W1.T * s_f[:, None]).astype(ml_dtypes.bfloat16))
    w3_h = np.ascontiguousarray((W3.T * s_f[:, None]).astype(ml_dtypes.bfloat16))
    w2_h = np.ascontiguousarray(W2.T.astype(ml_dtypes.bfloat16))
    bv_h = np.ascontiguousarray(bv[None, :])

    in_maps = []
    for b in range(B):
        in_maps.append({
            "lat": np.ascontiguousarray(latents[b]),
            "cb": codebook,
            "g2": np.ascontiguousarray(g2[b]),
            "cosq": cosq[b], "sinq": sinq[b],
            "wq": wq_h, "wk": wk_h, "wv": wv_h, "bv": bv_h, "wc": wc_h,
            "w1": w1_h, "w3": w3_h, "w2": w2_h,
        })
    return in_maps, tau, gns, rs, gum


def _host_post(results, tau, gns, gum):
    f32 = np.float32
    la_tau = np.stack([results[b]["latau"] for b in range(B)]).astype(f32)
    co = np.stack([results[b]["outp"] for b in range(B)]).astype(f32)
    la = (f32(tau) * la_tau - f32(gns) * gum).astype(f32)
    m = la_tau.max(axis=-1, keepdims=True)
    z = np.exp(la_tau - m, dtype=f32)
    z /= z.sum(axis=-1, keepdims=True)
    return co, la, la_tau.astype(f32), z.astype(f32)


def kernel(**inputs):
    in_maps, tau, gns, rs, gum = _host_prep(**inputs)
    nc = _get_program(rs)
    res = run_bass_kernel_spmd(nc, in_maps, core_ids=list(range(B)))
    return _host_post(res.results, tau, gns, gum)


def run_traced(**inputs):
    """test-only: same as kernel() but with NTFF tracing; returns (outs, res)."""
    in_maps, tau, gns, rs, gum = _host_prep(**inputs)
    nc = _get_program(rs)
    res = run_bass_kernel_spmd(nc, in_maps, core_ids=list(range(B)), trace=True)
    return _host_post(res.results, tau, gns, gum), res


# revision 18
# speedup vs baseline: 1.0967x; 1.0967x over previous
"""Trainium2 Bass kernel for nn_AttnCodebook (VQ codebook attention block).

Sharding: data-parallel over batch B=8 -> one batch element per NeuronCore.
Each core computes its batch element's attention + FF block; q/k/logits run
in a transposed activation layout, softmax rows in natural layout.

Device outputs per core: log_alpha_tau [N, C] f32 and codebook_output [N, D]
f32.  log_alpha and z are reconstructed on the host from log_alpha_tau
(exact linear relation / softmax), which halves the device DMA-out traffic.

Precision: float32r (TF32-like, full PE rate) for the rmsnorm->q/k->logits
chain; bf16 for the attention value path, Wc and the feed-forward.
"""

import math
import numpy as np

import concourse.bacc as bacc
import concourse.tile as tile
from concourse import mybir, masks
from concourse.bass_utils import run_bass_kernel_spmd
from contextlib import ExitStack

B, N, D, C, H = 8, 2048, 768, 2048, 3072
HALF = D // 2  # 384
ND = D // 128  # 6
NH = H // 128  # 24
NCC = C // 128  # 16
NNC = N // 128  # 16
NBLK = 4  # n-blocks of 512
BLK = N // NBLK  # 512
ROPE_BASE = 10000.0
EPS = 1e-6

dt = mybir.dt
AF = mybir.ActivationFunctionType
OP = mybir.AluOpType

F32, F32R, BF16 = dt.float32, dt.float32r, dt.bfloat16


def build_program(rs: float):
    """Build the per-core Bass program (same program for all 8 cores)."""
    nc = bacc.Bacc("TRN2", target_bir_lowering=False, debug=False, num_devices=1)

    # ---- DRAM I/O ----
    lat = nc.dram_tensor("lat", [N, D], F32, kind="ExternalInput").ap()
    cb = nc.dram_tensor("cb", [C, D], F32, kind="ExternalInput").ap()
    g2 = nc.dram_tensor("g2", [N, C], F32, kind="ExternalInput").ap()
    cosq = nc.dram_tensor("cosq", [HALF, N], F32, kind="ExternalInput").ap()
    sinq = nc.dram_tensor("sinq", [HALF, N], F32, kind="ExternalInput").ap()
    wq = nc.dram_tensor("wq", [D, D], F32R, kind="ExternalInput").ap()
    wk = nc.dram_tensor("wk", [D, D], F32R, kind="ExternalInput").ap()
    wv = nc.dram_tensor("wv", [D, D], F32R, kind="ExternalInput").ap()
    bv = nc.dram_tensor("bv", [1, D], F32R, kind="ExternalInput").ap()
    wc = nc.dram_tensor("wc", [D, D], BF16, kind="ExternalInput").ap()
    w1 = nc.dram_tensor("w1", [D, H], BF16, kind="ExternalInput").ap()
    w3 = nc.dram_tensor("w3", [D, H], BF16, kind="ExternalInput").ap()
    w2 = nc.dram_tensor("w2", [H, D], BF16, kind="ExternalInput").ap()

    latau = nc.dram_tensor("latau", [N, C], F32, kind="ExternalOutput").ap()
    outp = nc.dram_tensor("outp", [N, D], F32, kind="ExternalOutput").ap()
    aos = nc.dram_tensor("aos", [N, D], BF16, kind="ExternalOutput").ap()  # attn_out spill

    with tile.TileContext(nc) as tc, ExitStack() as octx:
        sbG = octx.enter_context(tc.tile_pool(name="glob", bufs=1))
        sbSc = octx.enter_context(tc.tile_pool(name="scal", bufs=4))
        psB = octx.enter_context(tc.tile_pool(name="psB", bufs=2, space="PSUM"))
        psAV = octx.enter_context(tc.tile_pool(name="psAV", bufs=1, space="PSUM"))
        psT = octx.enter_context(tc.tile_pool(name="psT", bufs=2, space="PSUM"))

        idf = sbG.tile([128, 128], F32, tag="idf")
        masks.make_identity(nc, idf[:])
        idb = sbG.tile([128, 128], BF16, tag="idb")
        masks.make_identity(nc, idb[:])
        ones_f = sbG.tile([1, 128], F32, tag="ones_f")
        nc.gpsimd.memset(ones_f[:], 1.0)
        ones_r = sbG.tile([1, 128], F32R, tag="ones_r")
        nc.vector.tensor_copy(ones_r[:], ones_f[:])
        eps_t = sbG.tile([128, 1], F32, tag="eps_t")
        nc.gpsimd.memset(eps_t[:], EPS)

        def rms_rinv(pool, x_t, nfree, tagp):
            """per-partition 1/sqrt(mean(x^2)+eps) of a [128, nfree] tile."""
            sq = pool.tile([128, nfree], F32, tag=f"sq{tagp}", bufs=1, name="sq")
            ss = sbSc.tile([128, 1], F32, tag=f"ss{tagp}", name="ss")
            nc.scalar.activation(sq[:], x_t[:], AF.Square, accum_out=ss[:])
            rms = sbSc.tile([128, 1], F32, tag=f"rms{tagp}", name="rms")
            nc.scalar.activation(rms[:], ss[:], AF.Sqrt, scale=1.0 / nfree,
                                 bias=eps_t[:])
            rinv = sbSc.tile([128, 1], F32, tag=f"rinv{tagp}", name="rinv")
            nc.vector.reciprocal(rinv[:], rms[:])
            return rinv

        def rope_pair(pool, ps_lo, ps_hi, cs_t, sn_t, out_lo, out_hi, w, tagp):
            """out_lo = lo*cos - hi*sin ; out_hi = lo*sin + hi*cos (width w)."""
            t1 = pool.tile([128, w], F32, tag=f"t1{tagp}", bufs=1, name="t1")
            t2 = pool.tile([128, w], F32, tag=f"t2{tagp}", bufs=1, name="t2")
            nc.vector.tensor_mul(t1[:], ps_lo, cs_t)
            nc.vector.tensor_mul(t2[:], ps_hi, sn_t)
            nc.vector.tensor_tensor(out_lo, t1[:], t2[:], OP.subtract)
            nc.vector.tensor_mul(t1[:], ps_lo, sn_t)
            nc.vector.tensor_mul(t2[:], ps_hi, cs_t)
            nc.vector.tensor_tensor(out_hi, t1[:], t2[:], OP.add)

        with tc.tile_pool(name="resA", bufs=1) as sbA:
            kT = [sbA.tile([128, C], F32R, tag=f"kT{j}", name=f"kT{j}")
                  for j in range(ND)]
            vT = [sbA.tile([128, D], BF16, tag=f"v{j}", name=f"v{j}")
                  for j in range(NCC)]
            wc_t = [sbA.tile([128, D], BF16, tag=f"wc{j}", name=f"wc{j}")
                    for j in range(ND)]
            wq_t = [sbA.tile([128, D], F32R, tag=f"wq{j}", name=f"wq{j}")
                    for j in range(ND)]

            # ===== PHASE 0: codebook -> kT (roped f32r), v (bf16), by C-half ====
            with tc.tile_pool(name="ph0", bufs=2) as sb0:
                wk_t = [sb0.tile([128, D], F32R, tag=f"wk{j}", bufs=1,
                                 name=f"wk{j}") for j in range(ND)]
                wv_t = [sb0.tile([128, D], F32R, tag=f"wv{j}", bufs=1,
                                 name=f"wv{j}") for j in range(ND)]
                for j in range(ND):
                    nc.sync.dma_start(wk_t[j][:], wk[j * 128:(j + 1) * 128, :])
                bv_t = sb0.tile([1, D], F32R, tag="bv", bufs=1)
                nc.sync.dma_start(bv_t[:], bv[:])

                for hh in range(2):
                    c0 = hh * 1024
                    ncsT = [sb0.tile([128, 1024], F32R, tag=f"ncsT{j}", bufs=1,
                                     name=f"ncsT{j}") for j in range(ND)]
                    for i in range(8):
                        ci = hh * 8 + i
                        cb_t = sb0.tile([128, D], F32, tag="cb", name="cb")
                        nc.sync.dma_start(cb_t[:], cb[ci * 128:(ci + 1) * 128, :])
                        rinv = rms_rinv(sb0, cb_t, D, "cb")
                        ncs = sb0.tile([128, D], F32, tag="ncs", name="ncs")
                        nc.vector.tensor_scalar_mul(ncs[:], cb_t[:], rinv[:])
                        for j in range(ND):
                            pt = psT.tile([128, 128], F32, tag="tp", name="pt")
                            nc.tensor.transpose(pt[:], ncs[:, j * 128:(j + 1) * 128],
                                                idf[:])
                            nc.scalar.copy(ncsT[j][:, i * 128:(i + 1) * 128], pt[:])
                    # k projection + rope for this half
                    for j in range(ND // 2):
                        pslo = psB.tile([128, 1024], F32, tag="mm", name="pslo")
                        pshi = psB.tile([128, 1024], F32, tag="mm", name="pshi")
                        for ps, jj in ((pslo, j), (pshi, j + ND // 2)):
                            for dk in range(ND):
                                for s in range(2):
                                    nc.tensor.matmul(
                                        ps[:, s * 512:(s + 1) * 512],
                                        wk_t[dk][:, jj * 128:(jj + 1) * 128],
                                        ncsT[dk][:, s * 512:(s + 1) * 512],
                                        start=(dk == 0), stop=(dk == ND - 1))
                        cs_t = sb0.tile([128, 1024], F32, tag="cosk", name="cs")
                        sn_t = sb0.tile([128, 1024], F32, tag="sink", name="sn")
                        nc.sync.dma_start(cs_t[:], cosq[j * 128:(j + 1) * 128,
                                                        c0:c0 + 1024])
                        nc.sync.dma_start(sn_t[:], sinq[j * 128:(j + 1) * 128,
                                                        c0:c0 + 1024])
                        rope_pair(sb0, pslo[:], pshi[:], cs_t[:], sn_t[:],
                                  kT[j][:, c0:c0 + 1024],
                                  kT[j + ND // 2][:, c0:c0 + 1024], 1024, "k")
                    # v for this half
                    if hh == 0:
                        for j in range(ND):
                            nc.sync.dma_start(wv_t[j][:], wv[j * 128:(j + 1) * 128, :])
                    for i in range(8):
                        ci = hh * 8 + i
                        vps = psAV.tile([128, D], F32, tag="av", name="vps")
                        for dk in range(ND):
                            for s0, sw in ((0, 512), (512, 256)):
                                nc.tensor.matmul(
                                    vps[:, s0:s0 + sw],
                                    ncsT[dk][:, i * 128:(i + 1) * 128],
                                    wv_t[dk][:, s0:s0 + sw],
                                    start=(dk == 0), stop=False)
                        for s0, sw in ((0, 512), (512, 256)):
                            nc.tensor.matmul(vps[:, s0:s0 + sw],
                                             ones_r[:], bv_t[:, s0:s0 + sw],
                                             start=False, stop=True)
                        nc.scalar.copy(vT[ci][:], vps[:, 0:D])

            # ======== PHASE A: queries + logits + attention, per n-block ========
            with tc.tile_pool(name="phA", bufs=2) as sbW, \
                 tc.tile_pool(name="phAq", bufs=1) as sbQ:
                pending = None
                lat_all = []
                for j in range(ND):
                    nc.sync.dma_start(wq_t[j][:], wq[j * 128:(j + 1) * 128, :])
                    nc.sync.dma_start(wc_t[j][:], wc[j * 128:(j + 1) * 128, :])
                for blk in range(NBLK):
                    # -- queries for this block --
                    lat_t = [sbW.tile([128, D], F32, tag=f"lat{k}", bufs=2,
                                      name=f"lat{k}") for k in range(4)]
                    lat_all.append(lat_t)
                    nqT = [sbQ.tile([128, BLK], F32R, tag=f"nqT{j}",
                                    name=f"nqT{j}") for j in range(ND)]
                    for k in range(4):
                        nch = blk * 4 + k
                        nc.sync.dma_start(lat_t[k][:],
                                          lat[nch * 128:(nch + 1) * 128, :])
                        rinv = rms_rinv(sbW, lat_t[k], D, "q")
                        nq = sbW.tile([128, D], F32, tag="nq", bufs=1, name="nq")
                        nc.vector.tensor_scalar_mul(nq[:], lat_t[k][:], rinv[:])
                        for j in range(ND):
                            pt = psT.tile([128, 128], F32, tag="tp", name="pt")
                            nc.tensor.transpose(pt[:], nq[:, j * 128:(j + 1) * 128],
                                                idf[:])
                            nc.scalar.copy(nqT[j][:, k * 128:(k + 1) * 128], pt[:])
                    qT = [sbQ.tile([128, BLK], F32R, tag=f"qT{j}", name=f"qT{j}")
                          for j in range(ND)]
                    for j in range(ND // 2):
                        pslo = psB.tile([128, 1024], F32, tag="mm", name="pslo")
                        pshi = psB.tile([128, 1024], F32, tag="mm", name="pshi")
                        for ps, jj in ((pslo, j), (pshi, j + ND // 2)):
                            for dk in range(ND):
                                nc.tensor.matmul(
                                    ps[:, 0:BLK],
                                    wq_t[dk][:, jj * 128:(jj + 1) * 128],
                                    nqT[dk][:], start=(dk == 0), stop=(dk == ND - 1))
                        cs_t = sbW.tile([128, BLK], F32, tag="coss", bufs=1,
                                        name="cs")
                        sn_t = sbW.tile([128, BLK], F32, tag="sins", bufs=1,
                                        name="sn")
                        nc.sync.dma_start(cs_t[:], cosq[j * 128:(j + 1) * 128,
                                                        blk * BLK:(blk + 1) * BLK])
                        nc.sync.dma_start(sn_t[:], sinq[j * 128:(j + 1) * 128,
                                                        blk * BLK:(blk + 1) * BLK])
                        rope_pair(sbW, pslo[:, 0:BLK], pshi[:, 0:BLK], cs_t[:],
                                  sn_t[:], qT[j][:], qT[j + ND // 2][:], BLK, "q")

                    # -- per n-chunk: logits then (pipelined) softmax+attention --
                    def emit_logits(nch, qT):
                        lt = sbW.tile([128, C], F32, tag="latau", name="lt")
                        kk = nch % 4
                        for hh in range(2):
                            g2_t = sbW.tile([128, 1024], F32, tag="g2", bufs=2,
                                            name="g2t")
                            nc.sync.dma_start(
                                g2_t[:], g2[nch * 128:(nch + 1) * 128,
                                            hh * 1024:(hh + 1) * 1024])
                            ps = psB.tile([128, 1024], F32, tag="mm", name="laps")
                            for dk in range(ND):
                                for s in range(2):
                                    cs0 = hh * 1024 + s * 512
                                    nc.tensor.matmul(
                                        ps[:, s * 512:(s + 1) * 512],
                                        qT[dk][:, kk * 128:(kk + 1) * 128],
                                        kT[dk][:, cs0:cs0 + 512],
                                        start=(dk == 0), stop=(dk == ND - 1))
                            nc.vector.scalar_tensor_tensor(
                                lt[:, hh * 1024:(hh + 1) * 1024], ps[:], 1.0,
                                g2_t[:], OP.mult, OP.add)
                        nc.sync.dma_start(latau[nch * 128:(nch + 1) * 128, :], lt[:])
                        # row-sum of exp via fused accumulate (e image unused)
                        e_bf = sbW.tile([128, C], BF16, tag="e", bufs=1, name="e_bf")
                        ssum = sbSc.tile([128, 1], F32, tag="ssum", name="ssum")
                        nc.scalar.activation(e_bf[:], lt[:], AF.Exp,
                                             accum_out=ssum[:])
                        rr = sbSc.tile([128, 1], F32, tag="rr", name="rr")
                        nc.vector.reciprocal(rr[:], ssum[:])
                        return nch, lt, rr

                    def emit_attn(state):
                        nch, lt, rr = state
                        kk = nch % 4
                        # eT tiles: transpose la_tau then exp on ACT (evacuates psT)
                        eTs = []
                        for ci in range(NCC):
                            pt = psT.tile([128, 128], F32, tag="tp", name="pt")
                            nc.tensor.transpose(pt[:], lt[:, ci * 128:(ci + 1) * 128],
                                                idf[:])
                            eT = sbW.tile([128, 128], BF16, tag=f"eT{ci}", bufs=1,
                                          name="eT")
                            nc.scalar.activation(eT[:], pt[:], AF.Exp)
                            eTs.append(eT)
                        aps = psAV.tile([128, D], F32, tag="av", name="aps")
                        for ci in range(NCC):
                            for s0, sw in ((0, 512), (512, 256)):
                                nc.tensor.matmul(aps[:, s0:s0 + sw], eTs[ci][:],
                                                 vT[ci][:, s0:s0 + sw],
                                                 start=(ci == 0), stop=(ci == NCC - 1))
                        at_bf = sbW.tile([128, D], BF16, tag="at", name="at")
                        nc.vector.tensor_scalar_mul(at_bf[:], aps[:, 0:D], rr[:])
                        yps = psAV.tile([128, D], F32, tag="av", name="yps")
                        aTs = []
                        for j in range(ND):
                            pt = psT.tile([128, 128], BF16, tag="tp", name="pt")
                            nc.tensor.transpose(pt[:], at_bf[:, j * 128:(j + 1) * 128],
                                                idb[:])
                            aT = sbW.tile([128, 128], BF16, tag=f"aT{j}", bufs=1,
                                          name="aT")
                            nc.scalar.copy(aT[:], pt[:])
                            aTs.append(aT)
                        for j in range(ND):
                            for s0, sw in ((0, 512), (512, 256)):
                                nc.tensor.matmul(yps[:, s0:s0 + sw], aTs[j][:],
                                                 wc_t[j][:, s0:s0 + sw],
                                                 start=(j == 0), stop=(j == ND - 1))
                        ao_bf = sbW.tile([128, D], BF16, tag="ao", name="ao")
                        nc.vector.scalar_tensor_tensor(ao_bf[:],
                                                       lat_all[nch // 4][nch % 4][:],
                                                       rs, yps[:, 0:D],
                                                       OP.mult, OP.add)
                        nc.sync.dma_start(aos[nch * 128:(nch + 1) * 128, :], ao_bf[:])

                    for k in range(4):
                        st = emit_logits(blk * 4 + k, qT)
                        if pending is not None:
                            emit_attn(pending)
                        pending = st
                if pending is not None:
                    emit_attn(pending)

        # ================= PHASE B: feed-forward =================
        with tc.tile_pool(name="phB", bufs=2) as sbB:
            # phase A writes `aos` via DRAM; Tile does not track DRAM RAW
            # deps across the spill, so order phases explicitly.
            tc.strict_bb_all_engine_barrier()

            def emit_w2(state):
                blkp, ao_p, p_p = state
                for k in range(4):
                    nch = blkp * 4 + k
                    fps = psAV.tile([128, D], F32, tag="av", name="fps")
                    for hs in range(NH):
                        for s0, sw in ((0, 512), (512, 256)):
                            nc.tensor.matmul(fps[:, s0:s0 + sw],
                                             p_p[hs][:, k * 128:(k + 1) * 128],
                                             w2keep[hs][:, s0:s0 + sw],
                                             start=(hs == 0), stop=(hs == NH - 1))
                    ot = sbB.tile([128, D], F32, tag="ot", name="ot")
                    nc.vector.scalar_tensor_tensor(ot[:], ao_p[k][:], 1.0,
                                                   fps[:, 0:D], OP.mult, OP.add)
                    nc.sync.dma_start(outp[nch * 128:(nch + 1) * 128, :], ot[:])

            # w2 is small (4.5MB bf16): keep resident, loaded once
            w2keep = [sbB.tile([128, D], BF16, tag=f"w2k{j}", bufs=1,
                               name=f"w2k{j}") for j in range(NH)]
            for j in range(NH):
                nc.sync.dma_start(w2keep[j][:], w2[j * 128:(j + 1) * 128, :])

            pendB = None
            for blk in range(NBLK):
                ao_t = [sbB.tile([128, D], BF16, tag=f"aob{k}", bufs=2,
                                 name=f"aob{k}") for k in range(4)]
                hT = [sbB.tile([128, BLK], BF16, tag=f"hT{j}", bufs=2,
                               name=f"hT{j}") for j in range(ND)]
                for k in range(4):
                    nch = blk * 4 + k
                    nc.sync.dma_start(ao_t[k][:], aos[nch * 128:(nch + 1) * 128, :])
                    rinv = rms_rinv(sbB, ao_t[k], D, "ff")
                    h_bf = sbB.tile([128, D], BF16, tag="h", name="h_bf")
                    nc.vector.tensor_scalar_mul(h_bf[:], ao_t[k][:], rinv[:])
                    for j in range(ND):
                        pt = psT.tile([128, 128], BF16, tag="tp", name="pt")
                        nc.tensor.transpose(pt[:], h_bf[:, j * 128:(j + 1) * 128],
                                            idb[:])
                        nc.scalar.copy(hT[j][:, k * 128:(k + 1) * 128], pt[:])
                p_t = [sbB.tile([128, BLK], BF16, tag=f"p{hs}", bufs=2,
                                name=f"p{hs}") for hs in range(NH)]
                w1r = w1.rearrange("(n p) m -> p n m", p=128)
                w3r = w3.rearrange("(n p) m -> p n m", p=128)
                for hp in range(NH // 2):
                    w1s = sbB.tile([128, ND, 256], BF16, tag="w1s", bufs=2,
                                   name="w1s")
                    w3s = sbB.tile([128, ND, 256], BF16, tag="w3s", bufs=2,
                                   name="w3s")
                    nc.sync.dma_start(w1s[:], w1r[:, :, hp * 256:(hp + 1) * 256])
                    nc.sync.dma_start(w3s[:], w3r[:, :, hp * 256:(hp + 1) * 256])
                    for hi in range(2):
                        hs = hp * 2 + hi
                        a1 = psB.tile([128, 1024], F32, tag="mm", name="a1")
                        for dk in range(ND):
                            nc.tensor.matmul(a1[:, 0:BLK],
                                             w1s[:, dk, hi * 128:(hi + 1) * 128],
                                             hT[dk][:], start=(dk == 0),
                                             stop=(dk == ND - 1))
                        a1s = sbB.tile([128, BLK], BF16, tag="a1s", name="a1s")
                        nc.scalar.activation(a1s[:], a1[:, 0:BLK], AF.Silu)
                        a3 = psB.tile([128, 1024], F32, tag="mm", name="a3")
                        for dk in range(ND):
                            nc.tensor.matmul(a3[:, 0:BLK],
                                             w3s[:, dk, hi * 128:(hi + 1) * 128],
                                             hT[dk][:], start=(dk == 0),
                                             stop=(dk == ND - 1))
                        nc.vector.tensor_tensor(p_t[hs][:], a3[:, 0:BLK], a1s[:],
                                                OP.mult)
                if pendB is not None:
                    emit_w2(pendB)
                pendB = (blk, ao_t, p_t)
            if pendB is not None:
                emit_w2(pendB)

    nc.compile()
    return nc


_PROG_CACHE = {}


def _get_program(rs: float):
    key = float(rs)
    if key not in _PROG_CACHE:
        _PROG_CACHE[key] = build_program(key)
    return _PROG_CACHE[key]


def _host_prep(latents, tau, residual_scaling, gumbel_noise_scale, positions,
               codebook, norm_context_scale, norm_queries_scale, norm_ff_scale,
               Wq, Wk, Wv, bv, Wc, W1, W2, W3):
    import ml_dtypes
    f32 = np.float32
    tau = float(np.asarray(tau))
    gns = float(np.asarray(gumbel_noise_scale))
    rs = float(np.asarray(residual_scaling))
    latents = np.asarray(latents, f32)
    positions = np.asarray(positions)
    codebook = np.ascontiguousarray(np.asarray(codebook, f32))
    s_q = np.asarray(norm_queries_scale, f32)
    s_c = np.asarray(norm_context_scale, f32)
    s_f = np.asarray(norm_ff_scale, f32)
    Wq = np.asarray(Wq, f32); Wk = np.asarray(Wk, f32); Wv = np.asarray(Wv, f32)
    bv = np.asarray(bv, f32); Wc = np.asarray(Wc, f32)
    W1 = np.asarray(W1, f32); W2 = np.asarray(W2, f32); W3 = np.asarray(W3, f32)

    # gumbel noise, bit-identical to the reference (jax threefry on CPU)
    import jax
    import jax.numpy as jnp
    cpu = jax.devices("cpu")[0]
    with jax.default_device(cpu):
        u = np.asarray(jax.random.uniform(jax.random.key(42), (B, N, C),
                                          dtype=jnp.float32))
    gum = -np.log(-np.log(u + f32(1e-10)) + f32(1e-10)).astype(f32)
    g2 = (f32(gns / tau) * gum).astype(f32)

    # rope tables, transposed per batch: [B, HALF, N]
    inv_freq = (1.0 / (ROPE_BASE ** (np.arange(HALF, dtype=f32) / f32(HALF)))).astype(f32)
    ang = positions[:, :N].astype(f32)[:, :, None] * inv_freq[None, None, :]
    cosq = np.ascontiguousarray(np.cos(ang).astype(f32).transpose(0, 2, 1))
    sinq = np.ascontiguousarray(np.sin(ang).astype(f32).transpose(0, 2, 1))

    # weight folds (host-side, exactly linear)
    wq_h = np.ascontiguousarray((Wq.T * (s_q[:, None] / f32(math.sqrt(D) * tau))).astype(f32))
    wk_h = np.ascontiguousarray((Wk.T * s_c[:, None]).astype(f32))
    wv_h = np.ascontiguousarray((Wv.T * s_c[:, None]).astype(f32))
    wc_h = np.ascontiguousarray(Wc.T.astype(ml_dtypes.bfloat16))
    w1_h = np.ascontiguousarray((W1.T * s_f[:, None]).astype(ml_dtypes.bfloat16))
    w3_h = np.ascontiguousarray((W3.T * s_f[:, None]).astype(ml_dtypes.bfloat16))
    w2_h = np.ascontiguousarray(W2.T.astype(ml_dtypes.bfloat16))
    bv_h = np.ascontiguousarray(bv[None, :])

    in_maps = []
    for b in range(B):
        in_maps.append({
            "lat": np.ascontiguousarray(latents[b]),
            "cb": codebook,
            "g2": np.ascontiguousarray(g2[b]),
            "cosq": cosq[b], "sinq": sinq[b],
            "wq": wq_h, "wk": wk_h, "wv": wv_h, "bv": bv_h, "wc": wc_h,
            "w1": w1_h, "w3": w3_h, "w2": w2_h,
        })
    return in_maps, tau, gns, rs, gum


def _host_post(results, tau, gns, gum):
    f32 = np.float32
    la_tau = np.stack([results[b]["latau"] for b in range(B)]).astype(f32)
    co = np.stack([results[b]["outp"] for b in range(B)]).astype(f32)
    la = (f32(tau) * la_tau - f32(gns) * gum).astype(f32)
    m = la_tau.max(axis=-1, keepdims=True)
    z = np.exp(la_tau - m, dtype=f32)
    z /= z.sum(axis=-1, keepdims=True)
    return co, la, la_tau.astype(f32), z.astype(f32)


def kernel(**inputs):
    in_maps, tau, gns, rs, gum = _host_prep(**inputs)
    nc = _get_program(rs)
    res = run_bass_kernel_spmd(nc, in_maps, core_ids=list(range(B)))
    return _host_post(res.results, tau, gns, gum)


def run_traced(**inputs):
    """test-only: same as kernel() but with NTFF tracing; returns (outs, res)."""
    in_maps, tau, gns, rs, gum = _host_prep(**inputs)
    nc = _get_program(rs)
    res = run_bass_kernel_spmd(nc, in_maps, core_ids=list(range(B)), trace=True)
    return _host_post(res.results, tau, gns, gum), res


# revision 19
# speedup vs baseline: 1.1168x; 1.0184x over previous
"""Trainium2 Bass kernel for nn_AttnCodebook (VQ codebook attention block).

Sharding: data-parallel over batch B=8 -> one batch element per NeuronCore.

Layout strategy: activations for the q/k/logits chain live transposed
([feature, token]); the logits are computed directly transposed
(laT = kT.T @ qT, [C, n]) so softmax exp tiles feed the attention matmul
with no on-chip transposes.  Row sums come from tiny ones-vector matmuls.
log_alpha_tau is written transposed and un-transposed on the host;
log_alpha and z are reconstructed on the host from log_alpha_tau.

Precision: float32r (TF32-like, full PE rate) for rmsnorm->q/k->logits;
bf16 for the attention value path, Wc and the feed-forward.
"""

import math
import numpy as np

import concourse.bacc as bacc
import concourse.tile as tile
from concourse import mybir, masks
from concourse.bass_utils import run_bass_kernel_spmd
from contextlib import ExitStack

B, N, D, C, H = 8, 2048, 768, 2048, 3072
HALF = D // 2  # 384
ND = D // 128  # 6
NH = H // 128  # 24
NCC = C // 128  # 16
NBLK = 4  # n-blocks of 512
BLK = N // NBLK  # 512
ROPE_BASE = 10000.0
EPS = 1e-6

dt = mybir.dt
AF = mybir.ActivationFunctionType
OP = mybir.AluOpType

F32, F32R, BF16 = dt.float32, dt.float32r, dt.bfloat16


def build_program(rs: float):
    """Build the per-core Bass program (same program for all 8 cores)."""
    nc = bacc.Bacc("TRN2", target_bir_lowering=False, debug=False, num_devices=1)

    # ---- DRAM I/O ----
    lat = nc.dram_tensor("lat", [N, D], F32, kind="ExternalInput").ap()
    cb = nc.dram_tensor("cb", [C, D], F32, kind="ExternalInput").ap()
    g2 = nc.dram_tensor("g2", [C, N], F32, kind="ExternalInput").ap()  # transposed
    cosq = nc.dram_tensor("cosq", [HALF, N], F32, kind="ExternalInput").ap()
    sinq = nc.dram_tensor("sinq", [HALF, N], F32, kind="ExternalInput").ap()
    wq = nc.dram_tensor("wq", [D, D], F32R, kind="ExternalInput").ap()
    wk = nc.dram_tensor("wk", [D, D], F32R, kind="ExternalInput").ap()
    wv = nc.dram_tensor("wv", [D, D], F32R, kind="ExternalInput").ap()
    bv = nc.dram_tensor("bv", [1, D], F32R, kind="ExternalInput").ap()
    wc = nc.dram_tensor("wc", [D, D], BF16, kind="ExternalInput").ap()
    w1 = nc.dram_tensor("w1", [D, H], BF16, kind="ExternalInput").ap()
    w3 = nc.dram_tensor("w3", [D, H], BF16, kind="ExternalInput").ap()
    w2 = nc.dram_tensor("w2", [H, D], BF16, kind="ExternalInput").ap()

    latau = nc.dram_tensor("latau", [C, N], F32, kind="ExternalOutput").ap()  # T
    outp = nc.dram_tensor("outp", [N, D], F32, kind="ExternalOutput").ap()
    aos = nc.dram_tensor("aos", [N, D], BF16, kind="ExternalOutput").ap()  # spill

    with tile.TileContext(nc) as tc, ExitStack() as octx:
        sbG = octx.enter_context(tc.tile_pool(name="glob", bufs=1))
        sbSc = octx.enter_context(tc.tile_pool(name="scal", bufs=4))
        psL = octx.enter_context(tc.tile_pool(name="psL", bufs=3, space="PSUM"))
        psAV = octx.enter_context(tc.tile_pool(name="psAV", bufs=1, space="PSUM"))
        psT = octx.enter_context(tc.tile_pool(name="psT", bufs=2, space="PSUM"))

        idf = sbG.tile([128, 128], F32, tag="idf")
        masks.make_identity(nc, idf[:])
        idb = sbG.tile([128, 128], BF16, tag="idb")
        masks.make_identity(nc, idb[:])
        ones_f = sbG.tile([1, 128], F32, tag="ones_f")
        nc.gpsimd.memset(ones_f[:], 1.0)
        ones_r = sbG.tile([1, 128], F32R, tag="ones_r")
        nc.vector.tensor_copy(ones_r[:], ones_f[:])
        onec_f = sbG.tile([128, 1], F32, tag="onec_f")
        nc.gpsimd.memset(onec_f[:], 1.0)
        onec_b = sbG.tile([128, 1], BF16, tag="onec_b")
        nc.vector.tensor_copy(onec_b[:], onec_f[:])
        eps_t = sbG.tile([128, 1], F32, tag="eps_t")
        nc.gpsimd.memset(eps_t[:], EPS)

        def rms_rinv(pool, x_t, nfree, tagp):
            """per-partition 1/sqrt(mean(x^2)+eps) of a [128, nfree] tile."""
            sq = pool.tile([128, nfree], F32, tag=f"sq{tagp}", bufs=1, name="sq")
            ss = sbSc.tile([128, 1], F32, tag=f"ss{tagp}", name="ss")
            nc.scalar.activation(sq[:], x_t[:], AF.Square, accum_out=ss[:])
            rms = sbSc.tile([128, 1], F32, tag=f"rms{tagp}", name="rms")
            nc.scalar.activation(rms[:], ss[:], AF.Sqrt, scale=1.0 / nfree,
                                 bias=eps_t[:])
            rinv = sbSc.tile([128, 1], F32, tag=f"rinv{tagp}", name="rinv")
            nc.vector.reciprocal(rinv[:], rms[:])
            return rinv

        def rope_pair(pool, ps_lo, ps_hi, cs_t, sn_t, out_lo, out_hi, w, tagp):
            """out_lo = lo*cos - hi*sin ; out_hi = lo*sin + hi*cos (width w)."""
            t1 = pool.tile([128, w], F32, tag=f"t1{tagp}", bufs=1, name="t1")
            t2 = pool.tile([128, w], F32, tag=f"t2{tagp}", bufs=1, name="t2")
            nc.vector.tensor_mul(t1[:], ps_lo, cs_t)
            nc.vector.tensor_mul(t2[:], ps_hi, sn_t)
            nc.vector.tensor_tensor(out_lo, t1[:], t2[:], OP.subtract)
            nc.vector.tensor_mul(t1[:], ps_lo, sn_t)
            nc.vector.tensor_mul(t2[:], ps_hi, cs_t)
            nc.vector.tensor_tensor(out_hi, t1[:], t2[:], OP.add)

        with tc.tile_pool(name="resA", bufs=1) as sbA:
            kT = [sbA.tile([128, C], F32R, tag=f"kT{j}", name=f"kT{j}")
                  for j in range(ND)]
            vT = [sbA.tile([128, D], BF16, tag=f"v{j}", name=f"v{j}")
                  for j in range(NCC)]
            wc_t = [sbA.tile([128, D], BF16, tag=f"wc{j}", name=f"wc{j}")
                    for j in range(ND)]
            wq_t = [sbA.tile([128, D], F32R, tag=f"wq{j}", name=f"wq{j}")
                    for j in range(ND)]

            # ===== PHASE 0: codebook -> kT (roped f32r), v (bf16), by C-half ====
            with tc.tile_pool(name="ph0", bufs=2) as sb0:
                wk_t = [sb0.tile([128, D], F32R, tag=f"wk{j}", bufs=1,
                                 name=f"wk{j}") for j in range(ND)]
                wv_t = [sb0.tile([128, D], F32R, tag=f"wv{j}", bufs=1,
                                 name=f"wv{j}") for j in range(ND)]
                for j in range(ND):
                    nc.sync.dma_start(wk_t[j][:], wk[j * 128:(j + 1) * 128, :])
                bv_t = sb0.tile([1, D], F32R, tag="bv", bufs=1)
                nc.sync.dma_start(bv_t[:], bv[:])

                for hh in range(2):
                    c0 = hh * 1024
                    ncsT = [sb0.tile([128, 1024], F32R, tag=f"ncsT{j}", bufs=1,
                                     name=f"ncsT{j}") for j in range(ND)]
                    for i in range(8):
                        ci = hh * 8 + i
                        cb_t = sb0.tile([128, D], F32, tag="cb", name="cb")
                        nc.sync.dma_start(cb_t[:], cb[ci * 128:(ci + 1) * 128, :])
                        rinv = rms_rinv(sb0, cb_t, D, "cb")
                        ncs = sb0.tile([128, D], F32, tag="ncs", name="ncs")
                        nc.vector.tensor_scalar_mul(ncs[:], cb_t[:], rinv[:])
                        for j in range(ND):
                            pt = psT.tile([128, 128], F32, tag="tp", name="pt")
                            nc.tensor.transpose(pt[:], ncs[:, j * 128:(j + 1) * 128],
                                                idf[:])
                            nc.scalar.copy(ncsT[j][:, i * 128:(i + 1) * 128], pt[:])
                    # k projection + rope for this half (512-wide quarters)
                    for j in range(ND // 2):
                        cs_t = sb0.tile([128, 1024], F32, tag="cosk", name="cs")
                        sn_t = sb0.tile([128, 1024], F32, tag="sink", name="sn")
                        nc.sync.dma_start(cs_t[:], cosq[j * 128:(j + 1) * 128,
                                                        c0:c0 + 1024])
                        nc.sync.dma_start(sn_t[:], sinq[j * 128:(j + 1) * 128,
                                                        c0:c0 + 1024])
                        for qq in range(2):
                            q0 = qq * 512
                            pslo = psL.tile([128, 512], F32, tag="mm", name="pslo")
                            pshi = psL.tile([128, 512], F32, tag="mm", name="pshi")
                            for ps, jj in ((pslo, j), (pshi, j + ND // 2)):
                                for dk in range(ND):
                                    nc.tensor.matmul(
                                        ps[:],
                                        wk_t[dk][:, jj * 128:(jj + 1) * 128],
                                        ncsT[dk][:, q0:q0 + 512],
                                        start=(dk == 0), stop=(dk == ND - 1))
                            rope_pair(sb0, pslo[:], pshi[:],
                                      cs_t[:, q0:q0 + 512], sn_t[:, q0:q0 + 512],
                                      kT[j][:, c0 + q0:c0 + q0 + 512],
                                      kT[j + ND // 2][:, c0 + q0:c0 + q0 + 512],
                                      512, "k")
                    # v for this half
                    if hh == 0:
                        for j in range(ND):
                            nc.sync.dma_start(wv_t[j][:], wv[j * 128:(j + 1) * 128, :])
                    for i in range(8):
                        ci = hh * 8 + i
                        vps = psAV.tile([128, D], F32, tag="av", name="vps")
                        for dk in range(ND):
                            for s0, sw in ((0, 512), (512, 256)):
                                nc.tensor.matmul(
                                    vps[:, s0:s0 + sw],
                                    ncsT[dk][:, i * 128:(i + 1) * 128],
                                    wv_t[dk][:, s0:s0 + sw],
                                    start=(dk == 0), stop=False)
                        for s0, sw in ((0, 512), (512, 256)):
                            nc.tensor.matmul(vps[:, s0:s0 + sw],
                                             ones_r[:], bv_t[:, s0:s0 + sw],
                                             start=False, stop=True)
                        nc.scalar.copy(vT[ci][:], vps[:, 0:D])

            # ======== PHASE A: queries + logits + attention, per n-block ========
            with tc.tile_pool(name="phA", bufs=2) as sbW, \
                 tc.tile_pool(name="phAq", bufs=1) as sbQ:
                for j in range(ND):
                    nc.sync.dma_start(wq_t[j][:], wq[j * 128:(j + 1) * 128, :])
                    nc.sync.dma_start(wc_t[j][:], wc[j * 128:(j + 1) * 128, :])

                def emit_queries(blk):
                    lat_t = [sbW.tile([128, D], F32, tag=f"lat{k}", bufs=2,
                                      name=f"lat{k}") for k in range(4)]
                    nqT = [sbQ.tile([128, BLK], F32R, tag=f"nqT{j}",
                                    name=f"nqT{j}") for j in range(ND)]
                    for k in range(4):
                        nch = blk * 4 + k
                        nc.sync.dma_start(lat_t[k][:],
                                          lat[nch * 128:(nch + 1) * 128, :])
                        rinv = rms_rinv(sbW, lat_t[k], D, "q")
                        nq = sbW.tile([128, D], F32, tag="nq", bufs=1, name="nq")
                        nc.vector.tensor_scalar_mul(nq[:], lat_t[k][:], rinv[:])
                        for j in range(ND):
                            pt = psT.tile([128, 128], F32, tag="tp", name="pt")
                            nc.tensor.transpose(pt[:], nq[:, j * 128:(j + 1) * 128],
                                                idf[:])
                            nc.scalar.copy(nqT[j][:, k * 128:(k + 1) * 128], pt[:])
                    qT = [sbQ.tile([128, BLK], F32R, tag=f"qT{j}", name=f"qT{j}")
                          for j in range(ND)]
                    for j in range(ND // 2):
                        pslo = psL.tile([128, 512], F32, tag="mm", name="pslo")
                        pshi = psL.tile([128, 512], F32, tag="mm", name="pshi")
                        for ps, jj in ((pslo, j), (pshi, j + ND // 2)):
                            for dk in range(ND):
                                nc.tensor.matmul(
                                    ps[:],
                                    wq_t[dk][:, jj * 128:(jj + 1) * 128],
                                    nqT[dk][:], start=(dk == 0), stop=(dk == ND - 1))
                        cs_t = sbW.tile([128, BLK], F32, tag="coss", bufs=1,
                                        name="cs")
                        sn_t = sbW.tile([128, BLK], F32, tag="sins", bufs=1,
                                        name="sn")
                        nc.sync.dma_start(cs_t[:], cosq[j * 128:(j + 1) * 128,
                                                        blk * BLK:(blk + 1) * BLK])
                        nc.sync.dma_start(sn_t[:], sinq[j * 128:(j + 1) * 128,
                                                        blk * BLK:(blk + 1) * BLK])
                        rope_pair(sbW, pslo[:], pshi[:], cs_t[:],
                                  sn_t[:], qT[j][:], qT[j + ND // 2][:], BLK, "q")
                    return lat_t, qT

                def emit_logits_blk(blk, qT):
                    """transposed logits + exp tiles + row sums for one n-block"""
                    n0 = blk * BLK
                    eTs = []
                    for ci in range(NCC):
                        ps = psL.tile([128, 512], F32, tag="mm", name="laps")
                        for dk in range(ND):
                            nc.tensor.matmul(ps[:],
                                             kT[dk][:, ci * 128:(ci + 1) * 128],
                                             qT[dk][:],
                                             start=(dk == 0), stop=(dk == ND - 1))
                        g2_t = sbW.tile([128, BLK], F32, tag="g2", bufs=3,
                                        name="g2t")
                        nc.sync.dma_start(g2_t[:],
                                          g2[ci * 128:(ci + 1) * 128, n0:n0 + BLK])
                        ltT = sbW.tile([128, BLK], F32, tag="ltT", bufs=3,
                                       name="ltT")
                        nc.vector.scalar_tensor_tensor(ltT[:], ps[:], 1.0,
                                                       g2_t[:], OP.mult, OP.add)
                        nc.sync.dma_start(latau[ci * 128:(ci + 1) * 128,
                                                n0:n0 + BLK], ltT[:])
                        eT = sbW.tile([128, BLK], BF16, tag=f"eT{ci}", bufs=1,
                                      name="eT")
                        nc.scalar.activation(eT[:], ltT[:], AF.Exp)
                        eTs.append(eT)
                    # per-chunk row sums: ssum[n] = sum_c eT[c, n]
                    rrs = []
                    for k in range(4):
                        cps = psT.tile([128, 1], F32, tag="csum", bufs=1,
                                       name="cps")
                        for ci in range(NCC):
                            nc.tensor.matmul(cps[:],
                                             eTs[ci][:, k * 128:(k + 1) * 128],
                                             onec_b[:],
                                             start=(ci == 0), stop=(ci == NCC - 1))
                        rr = sbSc.tile([128, 1], F32, tag="rr", name="rr")
                        nc.vector.reciprocal(rr[:], cps[:])
                        rrs.append(rr)
                    return eTs, rrs

                def emit_attn_blk(state):
                    blkp, lat_t, eTs, rrs = state
                    for k in range(4):
                        nch = blkp * 4 + k
                        aps = psAV.tile([128, D], F32, tag="av", name="aps")
                        for ci in range(NCC):
                            for s0, sw in ((0, 512), (512, 256)):
                                nc.tensor.matmul(
                                    aps[:, s0:s0 + sw],
                                    eTs[ci][:, k * 128:(k + 1) * 128],
                                    vT[ci][:, s0:s0 + sw],
                                    start=(ci == 0), stop=(ci == NCC - 1))
                        at_bf = sbW.tile([128, D], BF16, tag="at", name="at")
                        nc.scalar.copy(at_bf[:], aps[:, 0:D])
                        yps = psAV.tile([128, D], F32, tag="av", name="yps")
                        aTs = []
                        for j in range(ND):
                            pt = psT.tile([128, 128], BF16, tag="tp", name="pt")
                            nc.tensor.transpose(pt[:],
                                                at_bf[:, j * 128:(j + 1) * 128],
                                                idb[:])
                            aT = sbW.tile([128, 128], BF16, tag=f"aT{j}", bufs=1,
                                          name="aT")
                            nc.scalar.copy(aT[:], pt[:])
                            aTs.append(aT)
                        for j in range(ND):
                            for s0, sw in ((0, 512), (512, 256)):
                                nc.tensor.matmul(yps[:, s0:s0 + sw], aTs[j][:],
                                                 wc_t[j][:, s0:s0 + sw],
                                                 start=(j == 0), stop=(j == ND - 1))
                        ao_bf = sbW.tile([128, D], BF16, tag="ao", name="ao")
                        if rs == 1.0:
                            lat_in = lat_t[k][:]
                        else:
                            lrs = sbW.tile([128, D], F32, tag="latrs", bufs=2,
                                           name="latrs")
                            nc.scalar.mul(lrs[:], lat_t[k][:], rs)
                            lat_in = lrs[:]
                        nc.vector.scalar_tensor_tensor(ao_bf[:], yps[:, 0:D],
                                                       rrs[k][:], lat_in,
                                                       OP.mult, OP.add)
                        nc.sync.dma_start(aos[nch * 128:(nch + 1) * 128, :],
                                          ao_bf[:])

                pending = None
                for blk in range(NBLK):
                    lat_t, qT = emit_queries(blk)
                    eTs, rrs = emit_logits_blk(blk, qT)
                    if pending is not None:
                        emit_attn_blk(pending)
                    pending = (blk, lat_t, eTs, rrs)
                if pending is not None:
                    emit_attn_blk(pending)

        # ================= PHASE B: feed-forward =================
        with tc.tile_pool(name="phB", bufs=2) as sbB:
            # phase A writes `aos` via DRAM; Tile does not track DRAM RAW
            # deps across the spill, so order phases explicitly.
            tc.strict_bb_all_engine_barrier()

            def emit_w2(state):
                blkp, ao_p, p_p = state
                for k in range(4):
                    nch = blkp * 4 + k
                    fps = psAV.tile([128, D], F32, tag="av", name="fps")
                    for hs in range(NH):
                        for s0, sw in ((0, 512), (512, 256)):
                            nc.tensor.matmul(fps[:, s0:s0 + sw],
                                             p_p[hs][:, k * 128:(k + 1) * 128],
                                             w2keep[hs][:, s0:s0 + sw],
                                             start=(hs == 0), stop=(hs == NH - 1))
                    ot = sbB.tile([128, D], F32, tag="ot", name="ot")
                    nc.vector.scalar_tensor_tensor(ot[:], ao_p[k][:], 1.0,
                                                   fps[:, 0:D], OP.mult, OP.add)
                    nc.sync.dma_start(outp[nch * 128:(nch + 1) * 128, :], ot[:])

            # w2 is small (4.5MB bf16): keep resident, loaded once
            w2keep = [sbB.tile([128, D], BF16, tag=f"w2k{j}", bufs=1,
                               name=f"w2k{j}") for j in range(NH)]
            for j in range(NH):
                nc.sync.dma_start(w2keep[j][:], w2[j * 128:(j + 1) * 128, :])

            w1r = w1.rearrange("(n p) m -> p n m", p=128)
            w3r = w3.rearrange("(n p) m -> p n m", p=128)
            pendB = None
            for blk in range(NBLK):
                ao_t = [sbB.tile([128, D], BF16, tag=f"aob{k}", bufs=2,
                                 name=f"aob{k}") for k in range(4)]
                hT = [sbB.tile([128, BLK], BF16, tag=f"hT{j}", bufs=2,
                               name=f"hT{j}") for j in range(ND)]
                for k in range(4):
                    nch = blk * 4 + k
                    nc.sync.dma_start(ao_t[k][:], aos[nch * 128:(nch + 1) * 128, :])
                    rinv = rms_rinv(sbB, ao_t[k], D, "ff")
                    h_bf = sbB.tile([128, D], BF16, tag="h", name="h_bf")
                    nc.vector.tensor_scalar_mul(h_bf[:], ao_t[k][:], rinv[:])
                    for j in range(ND):
                        pt = psT.tile([128, 128], BF16, tag="tp", name="pt")
                        nc.tensor.transpose(pt[:], h_bf[:, j * 128:(j + 1) * 128],
                                            idb[:])
                        nc.scalar.copy(hT[j][:, k * 128:(k + 1) * 128], pt[:])
                p_t = [sbB.tile([128, BLK], BF16, tag=f"p{hs}", bufs=2,
                                name=f"p{hs}") for hs in range(NH)]
                for hp in range(NH // 2):
                    w1s = sbB.tile([128, ND, 256], BF16, tag="w1s", bufs=2,
                                   name="w1s")
                    w3s = sbB.tile([128, ND, 256], BF16, tag="w3s", bufs=2,
                                   name="w3s")
                    nc.sync.dma_start(w1s[:], w1r[:, :, hp * 256:(hp + 1) * 256])
                    nc.sync.dma_start(w3s[:], w3r[:, :, hp * 256:(hp + 1) * 256])
                    for hi in range(2):
                        hs = hp * 2 + hi
                        a1 = psL.tile([128, 512], F32, tag="mm", name="a1")
                        for dk in range(ND):
                            nc.tensor.matmul(a1[:],
                                             w1s[:, dk, hi * 128:(hi + 1) * 128],
                                             hT[dk][:], start=(dk == 0),
                                             stop=(dk == ND - 1))
                        a1s = sbB.tile([128, BLK], BF16, tag="a1s", name="a1s")
                        nc.scalar.activation(a1s[:], a1[:], AF.Silu)
                        a3 = psL.tile([128, 512], F32, tag="mm", name="a3")
                        for dk in range(ND):
                            nc.tensor.matmul(a3[:],
                                             w3s[:, dk, hi * 128:(hi + 1) * 128],
                                             hT[dk][:], start=(dk == 0),
                                             stop=(dk == ND - 1))
                        nc.vector.tensor_tensor(p_t[hs][:], a3[:], a1s[:],
                                                OP.mult)
                if pendB is not None:
                    emit_w2(pendB)
                pendB = (blk, ao_t, p_t)
            if pendB is not None:
                emit_w2(pendB)

    nc.compile()
    return nc


_PROG_CACHE = {}


def _get_program(rs: float):
    key = float(rs)
    if key not in _PROG_CACHE:
        _PROG_CACHE[key] = build_program(key)
    return _PROG_CACHE[key]


def _host_prep(latents, tau, residual_scaling, gumbel_noise_scale, positions,
               codebook, norm_context_scale, norm_queries_scale, norm_ff_scale,
               Wq, Wk, Wv, bv, Wc, W1, W2, W3):
    import ml_dtypes
    f32 = np.float32
    tau = float(np.asarray(tau))
    gns = float(np.asarray(gumbel_noise_scale))
    rs = float(np.asarray(residual_scaling))
    latents = np.asarray(latents, f32)
    positions = np.asarray(positions)
    codebook = np.ascontiguousarray(np.asarray(codebook, f32))
    s_q = np.asarray(norm_queries_scale, f32)
    s_c = np.asarray(norm_context_scale, f32)
    s_f = np.asarray(norm_ff_scale, f32)
    Wq = np.asarray(Wq, f32); Wk = np.asarray(Wk, f32); Wv = np.asarray(Wv, f32)
    bv = np.asarray(bv, f32); Wc = np.asarray(Wc, f32)
    W1 = np.asarray(W1, f32); W2 = np.asarray(W2, f32); W3 = np.asarray(W3, f32)

    # gumbel noise, bit-identical to the reference (jax threefry on CPU)
    import jax
    import jax.numpy as jnp
    cpu = jax.devices("cpu")[0]
    with jax.default_device(cpu):
        u = np.asarray(jax.random.uniform(jax.random.key(42), (B, N, C),
                                          dtype=jnp.float32))
    gum = -np.log(-np.log(u + f32(1e-10)) + f32(1e-10)).astype(f32)
    g2 = (f32(gns / tau) * gum).astype(f32)

    # rope tables, transposed per batch: [B, HALF, N]
    inv_freq = (1.0 / (ROPE_BASE ** (np.arange(HALF, dtype=f32) / f32(HALF)))).astype(f32)
    ang = positions[:, :N].astype(f32)[:, :, None] * inv_freq[None, None, :]
    cosq = np.ascontiguousarray(np.cos(ang).astype(f32).transpose(0, 2, 1))
    sinq = np.ascontiguousarray(np.sin(ang).astype(f32).transpose(0, 2, 1))

    # weight folds (host-side, exactly linear)
    wq_h = np.ascontiguousarray((Wq.T * (s_q[:, None] / f32(math.sqrt(D) * tau))).astype(f32))
    wk_h = np.ascontiguousarray((Wk.T * s_c[:, None]).astype(f32))
    wv_h = np.ascontiguousarray((Wv.T * s_c[:, None]).astype(f32))
    wc_h = np.ascontiguousarray(Wc.T.astype(ml_dtypes.bfloat16))
    w1_h = np.ascontiguousarray((W1.T * s_f[:, None]).astype(ml_dtypes.bfloat16))
    w3_h = np.ascontiguousarray((W3.T * s_f[:, None]).astype(ml_dtypes.bfloat16))
    w2_h = np.ascontiguousarray(W2.T.astype(ml_dtypes.bfloat16))
    bv_h = np.ascontiguousarray(bv[None, :])

    in_maps = []
    for b in range(B):
        in_maps.append({
            "lat": np.ascontiguousarray(latents[b]),
            "cb": codebook,
            "g2": np.ascontiguousarray(g2[b].T),
            "cosq": cosq[b], "sinq": sinq[b],
            "wq": wq_h, "wk": wk_h, "wv": wv_h, "bv": bv_h, "wc": wc_h,
            "w1": w1_h, "w3": w3_h, "w2": w2_h,
        })
    return in_maps, tau, gns, rs, gum


def _host_post(results, tau, gns, gum):
    f32 = np.float32
    la_tau = np.stack([np.asarray(results[b]["latau"]).T for b in range(B)])
    la_tau = np.ascontiguousarray(la_tau).astype(f32, copy=False)
    co = np.stack([results[b]["outp"] for b in range(B)]).astype(f32)
    la = (f32(tau) * la_tau - f32(gns) * gum).astype(f32)
    m = la_tau.max(axis=-1, keepdims=True)
    z = np.exp(la_tau - m, dtype=f32)
    z /= z.sum(axis=-1, keepdims=True)
    return co, la, la_tau, z.astype(f32)


def kernel(**inputs):
    in_maps, tau, gns, rs, gum = _host_prep(**inputs)
    nc = _get_program(rs)
    res = run_bass_kernel_spmd(nc, in_maps, core_ids=list(range(B)))
    return _host_post(res.results, tau, gns, gum)


def run_traced(**inputs):
    """test-only: same as kernel() but with NTFF tracing; returns (outs, res)."""
    in_maps, tau, gns, rs, gum = _host_prep(**inputs)
    nc = _get_program(rs)
    res = run_bass_kernel_spmd(nc, in_maps, core_ids=list(range(B)), trace=True)
    return _host_post(res.results, tau, gns, gum), res
